# revision 7
# baseline (speedup 1.0000x reference)
"""HRN routing kernel for Trainium2 (8 NeuronCores, Bass/Tile).

Strategy:
  - Wt (12288x12288, 604MB) is column-sharded across 8 cores. Each step every
    core computes y_shard = xf @ Wt[:, shard] streaming its 75.5MB shard from
    HBM (the memory-bound inner loop), then tanh(+bias), AllGather of the
    transposed activation shard rebuilds the full xf on every core.
  - The hash h = xf2 @ P uses a row-shard of P (resident in SBUF); partial
    hashes are AllGathered and summed locally.
  - Routing (projection magnitudes over 32 units) is sharded 4 units/core;
    the per-unit squared magnitudes use precomputed Gram matrices G_u =
    B_u^T B_u; a tiny AllGather rebuilds the full (16,32) magnitude table on
    every core, argmax/masking is replicated, and the selected-unit residual
    is computed by the owning core only and AllReduced at the end.
  - Step 15 needs routing only (its xf2/h are discarded by the reference).
"""

import numpy as np

B = 16
D = 12288
HASH = 1024
KB = 128
NU = 32
DEPTH = 16
NCORE = 8
DSH = D // NCORE          # 1536 columns of Wt per core
NKT = D // 128            # 96 k-tiles
NKL = DSH // 128          # 12 local k-tiles
NKP = NKT // 2            # 48 paired k-tiles (2 per DMA)
USH = NU // NCORE         # 4 units per core
FB = HASH // 128          # 8 f-blocks

_PROGRAM = None
LAST_OUT = {}


def _build_program():
    import concourse.bass as bass
    import concourse.mybir as mybir
    import concourse.tile as tile
    from concourse import bacc

    f32 = mybir.dt.float32
    i32 = mybir.dt.int32
    AF = mybir.ActivationFunctionType
    ALU = mybir.AluOpType

    nc = bacc.Bacc(
        "TRN2",
        target_bir_lowering=False,
        debug=False,
        enable_asserts=False,
        num_devices=NCORE,
    )

    # ---- I/O ----
    xT_in = nc.dram_tensor("xT", [128, NKT, B], f32, kind="ExternalInput").ap()
    xo_in = nc.dram_tensor("xo", [128, NKL, B], f32, kind="ExternalInput").ap()
    wt_in = nc.dram_tensor("wt", [NKP, 128, 2 * DSH], f32, kind="ExternalInput").ap()
    p_in = nc.dram_tensor("pp", [128, NKL, HASH], f32, kind="ExternalInput").ap()
    b_in = nc.dram_tensor("bb", [NU, DSH], f32, kind="ExternalInput").ap()
    bas_in = nc.dram_tensor("bas", [128, USH, FB, 128], f32, kind="ExternalInput").ap()
    basT_in = nc.dram_tensor("basT", [128, USH, FB, 128], f32, kind="ExternalInput").ap()
    g_in = nc.dram_tensor("gg", [128, USH, 128], f32, kind="ExternalInput").ap()
    usel_in = nc.dram_tensor("usel", [NU, USH], f32, kind="ExternalInput").ap()
    iota_in = nc.dram_tensor("iota", [B, NU], f32, kind="ExternalInput").ap()
    ident_in = nc.dram_tensor("ident", [128, 128], f32, kind="ExternalInput").ap()

    dbg_mags_out = nc.dram_tensor("dbg_mags", [B, DEPTH * NU], f32, kind="ExternalOutput").ap()
    dbg_h0_out = nc.dram_tensor("dbg_h0", [128, FB * B], f32, kind="ExternalOutput").ap()
    hashes_out = nc.dram_tensor("hashes", [B, HASH], f32, kind="ExternalOutput").ap()
    routes_out = nc.dram_tensor("routes", [B, DEPTH], i32, kind="ExternalOutput").ap()

    RG = [list(range(NCORE))]

    with tile.TileContext(nc) as tc:
        with (
            tc.tile_pool(name="const", bufs=1) as cpool,
            tc.tile_pool(name="wtp", bufs=4) as wtp,
            tc.tile_pool(name="xfp", bufs=2) as xfp,
            tc.tile_pool(name="work", bufs=2) as work,
            tc.tile_pool(name="ps_y", bufs=1, space="PSUM") as ps_y,
            tc.tile_pool(name="ps_small", bufs=2, space="PSUM") as ps_small,
            tc.tile_pool(name="ps_x2t", bufs=2, space="PSUM") as ps_x2t,
            tc.tile_pool(name="dram", bufs=2, space="DRAM") as dram,
        ):
            # ---- constants into SBUF ----
            p_sb = cpool.tile([128, NKL * HASH], f32)
            nc.sync.dma_start(p_sb, p_in.rearrange("p a b -> p (a b)"))
            bas_sb = cpool.tile([128, USH * FB * 128], f32)
            nc.sync.dma_start(bas_sb, bas_in.rearrange("p a b c -> p (a b c)"))
            basT_sb = cpool.tile([128, USH * FB * 128], f32)
            nc.sync.dma_start(basT_sb, basT_in.rearrange("p a b c -> p (a b c)"))
            g_sb = cpool.tile([128, USH * 128], f32)
            nc.sync.dma_start(g_sb, g_in.rearrange("p a b -> p (a b)"))
            b_sb = cpool.tile([NU, DSH], f32)
            nc.sync.dma_start(b_sb, b_in)
            usel_sb = cpool.tile([NU, USH], f32)
            nc.sync.dma_start(usel_sb, usel_in)
            iota_sb = cpool.tile([B, NU], f32)
            nc.sync.dma_start(iota_sb, iota_in)
            ident_sb = cpool.tile([128, 128], f32)
            nc.sync.dma_start(ident_sb, ident_in)
            xo_sb = cpool.tile([128, NKL * B], f32)
            nc.sync.dma_start(xo_sb, xo_in.rearrange("p a b -> p (a b)"))
            ones_sb = cpool.tile([128, 1], f32)
            nc.vector.memset(ones_sb, 1.0)
            ones1_sb = cpool.tile([1, 128], f32)
            nc.vector.memset(ones1_sb, 1.0)
            maskacc_sb = cpool.tile([B, NU], f32)
            nc.vector.memset(maskacc_sb, 0.0)
            hacc_sb = cpool.tile([128, FB * B], f32)
            nc.vector.memset(hacc_sb, 0.0)
            routes_f_sb = cpool.tile([B, DEPTH], f32)
            dbg_mags_sb = cpool.tile([B, DEPTH * NU], f32)

            # ---- initial xf ----
            xf_sb = xfp.tile([128, NKT * B], f32, tag="xf", name="xf0")
            nc.sync.dma_start(xf_sb, xT_in.rearrange("p a b -> p (a b)"))

            def h_partial_and_gather(x2t_ap, h_ps, step):
                """96 matmuls: h_partial[f,i] = sum_d P_sh[d,f] * x2t[d,i];
                then AllGather partials and tree-sum into a fresh hT tile."""
                # fb-outer so each fb's accumulation group is consecutive:
                # start=True clears has_written for the WHOLE psum bank, so a
                # later group's start must not interleave an open group.
                for fb in range(FB):
                    for ktl in range(NKL):
                        nc.tensor.matmul(
                            h_ps[:, fb * B:(fb + 1) * B],
                            lhsT=p_sb[:, (ktl * FB + fb) * 128:(ktl * FB + fb + 1) * 128],
                            rhs=x2t_ap[:, ktl * B:(ktl + 1) * B],
                            start=(ktl == 0),
                            stop=(ktl == NKL - 1),
                        )
                hp_sb = work.tile([128, FB * B], f32, tag="hp", name=f"hp{step}")
                nc.vector.tensor_copy(hp_sb, h_ps)
                agh_in = dram.tile([128, FB * B], f32, tag="aghi", name=f"aghi{step}")
                nc.sync.dma_start(agh_in, hp_sb)
                agh_out = dram.tile(
                    [NCORE * 128, FB * B], f32, tag="agho", name=f"agho{step}",
                    addr_space="Shared",
                )
                nc.gpsimd.collective_compute(
                    "AllGather", mybir.AluOpType.bypass,
                    replica_groups=RG, ins=[agh_in.opt()], outs=[agh_out.opt()],
                )
                hr_sb = work.tile([128, NCORE * FB * B], f32, tag="hr", name=f"hr{step}")
                nc.sync.dma_start(hr_sb, agh_out.rearrange("(r p) c -> p r c", p=128))
                hT = work.tile([128, FB * B], f32, tag="hT", name=f"hT{step}")
                nc.vector.tensor_add(hT, hr_sb[:, 0:FB * B], hr_sb[:, FB * B:2 * FB * B])
                for r in range(2, NCORE):
                    nc.vector.tensor_add(
                        hT, hT, hr_sb[:, r * FB * B:(r + 1) * FB * B]
                    )
                return hT

            # ---- h0 (prologue) ----
            sm0 = ps_small.tile([128, 512], f32, tag="sm", name="sm_pro")
            hT_cur = h_partial_and_gather(xo_sb, sm0[:, 356:356 + FB * B], "pro")
            nc.sync.dma_start(dbg_h0_out, hT_cur)

            for t in range(DEPTH):
                last = t == DEPTH - 1
                sm = ps_small.tile([128, 512], f32, tag="sm", name=f"sm{t}")
                c_ps = sm[:, 0:USH * B]
                gc_ps = sm[:, 64:64 + USH * B]
                m2_ps = sm[0:B, 128:128 + USH]
                oh_ps = sm[0:NU, 132:132 + B]
                ow_ps = sm[0:1, 148:148 + USH * B]
                bc_ps = sm[:, 164:164 + USH * B]
                pr_ps = sm[:, 228:228 + FB * B]
                h_ps = sm[:, 356:356 + FB * B]

                if not last:
                    y_ps = ps_y.tile([B, DSH], f32, tag="y", name=f"y{t}")

                def y_chunk(kp0, kp1):
                    for kp in range(kp0, kp1):
                        w_t = wtp.tile([128, 2 * DSH], f32, tag="wt", name=f"w{t}_{kp}")
                        nc.sync.dma_start(w_t, wt_in[kp])
                        for kte in range(2):
                            kt = kp * 2 + kte
                            for c in range(3):
                                nc.tensor.matmul(
                                    y_ps[:, c * 512:(c + 1) * 512],
                                    lhsT=xf_sb[:, kt * B:(kt + 1) * B],
                                    rhs=w_t[:, kte * DSH + c * 512:kte * DSH + (c + 1) * 512],
                                    start=(kt == 0),
                                    stop=False,
                                )

                if not last:
                    y_chunk(0, 16)

                # ---- routing part A: coeff, Gram magnitudes, mags AllGather ----
                for u in range(USH):
                    for fb in range(FB):
                        nc.tensor.matmul(
                            c_ps[:, u * B:(u + 1) * B],
                            lhsT=bas_sb[:, (u * FB + fb) * 128:(u * FB + fb + 1) * 128],
                            rhs=hT_cur[:, fb * B:(fb + 1) * B],
                            start=(fb == 0),
                            stop=(fb == FB - 1),
                        )
                coeff_sb = work.tile([128, USH * B], f32, tag="coeff", name=f"co{t}")
                nc.vector.tensor_copy(coeff_sb, c_ps)
                for u in range(USH):
                    nc.tensor.matmul(
                        gc_ps[:, u * B:(u + 1) * B],
                        lhsT=g_sb[:, u * 128:(u + 1) * 128],
                        rhs=coeff_sb[:, u * B:(u + 1) * B],
                        start=True, stop=True,
                    )
                tmp_sb = work.tile([128, USH * B], f32, tag="tmp", name=f"tm{t}")
                nc.vector.tensor_mul(tmp_sb, coeff_sb, gc_ps)
                for u in range(USH):
                    nc.tensor.matmul(
                        m2_ps[:, u:u + 1],
                        lhsT=tmp_sb[:, u * B:(u + 1) * B],
                        rhs=ones_sb[:, 0:1],
                        start=True, stop=True,
                    )
                m2_sb = work.tile([B, USH], f32, tag="m2", name=f"m2{t}")
                nc.vector.tensor_copy(m2_sb, m2_ps)
                agm_in = dram.tile([B, USH], f32, tag="agmi", name=f"agmi{t}")
                nc.sync.dma_start(agm_in, m2_sb)
                agm_out = dram.tile(
                    [NCORE * B, USH], f32, tag="agmo", name=f"agmo{t}",
                    addr_space="Shared",
                )
                nc.gpsimd.collective_compute(
                    "AllGather", mybir.AluOpType.bypass,
                    replica_groups=RG, ins=[agm_in.opt()], outs=[agm_out.opt()],
                )
                mags_sb = work.tile([B, NU], f32, tag="mags", name=f"mg{t}")
                nc.sync.dma_start(
                    mags_sb, agm_out.rearrange("(r i) u -> i r u", i=B)
                )

                if not last:
                    y_chunk(16, NKP)

                # ---- routing part B: argmax, onehot, residual ----
                nc.vector.tensor_copy(dbg_mags_sb[:, t * NU:(t + 1) * NU], mags_sb)
                masked_sb = work.tile([B, NU], f32, tag="masked", name=f"mk{t}")
                nc.vector.tensor_add(masked_sb, mags_sb, maskacc_sb)
                mx_sb = work.tile([B, 1], f32, tag="mx", name=f"mx{t}")
                nc.vector.reduce_max(mx_sb, masked_sb, axis=mybir.AxisListType.X)
                oh_sb = work.tile([B, NU], f32, tag="oh", name=f"oh{t}")
                nc.vector.tensor_scalar(
                    oh_sb, masked_sb, mx_sb, None, op0=ALU.is_equal
                )
                # maskacc -= 1e30 * onehot
                nc.vector.scalar_tensor_tensor(
                    maskacc_sb, oh_sb, -1e30, maskacc_sb,
                    op0=ALU.mult, op1=ALU.add,
                )
                # routes column t = sum(onehot * iota)
                ridx_sb = work.tile([B, NU], f32, tag="ridx", name=f"ri{t}")
                nc.vector.tensor_mul(ridx_sb, oh_sb, iota_sb)
                nc.vector.reduce_sum(routes_f_sb[:, t:t + 1], ridx_sb, axis=mybir.AxisListType.X)
                # onehotP (32, 16) via PE transpose
                nc.tensor.transpose(oh_ps, oh_sb, ident_sb[0:B, 0:B])
                ohp_sb = work.tile([NU, B], f32, tag="ohp", name=f"op{t}")
                nc.vector.tensor_copy(ohp_sb, oh_ps)
                # own-unit rows of onehotP, each to partition 0: (1, 16) x 4
                for u in range(USH):
                    nc.tensor.matmul(
                        ow_ps[:, u * B:(u + 1) * B],
                        lhsT=usel_sb[:, u:u + 1],
                        rhs=ohp_sb,
                        start=True, stop=True,
                    )
                own_sb = work.tile([1, USH * B], f32, tag="own", name=f"ow{t}")
                nc.vector.tensor_copy(own_sb, ow_ps)
                # broadcast own rows across 128 partitions
                for u in range(USH):
                    nc.tensor.matmul(
                        bc_ps[:, u * B:(u + 1) * B],
                        lhsT=ones1_sb,
                        rhs=own_sb[0:1, u * B:(u + 1) * B],
                        start=True, stop=True,
                    )
                csel_sb = work.tile([128, USH * B], f32, tag="csel", name=f"cs{t}")
                nc.vector.tensor_mul(csel_sb, coeff_sb, bc_ps)
                # selected projection (own units only), accumulated over units
                for fb in range(FB):
                    for u in range(USH):
                        nc.tensor.matmul(
                            pr_ps[:, fb * B:(fb + 1) * B],
                            lhsT=basT_sb[:, (u * FB + fb) * 128:(u * FB + fb + 1) * 128],
                            rhs=csel_sb[:, u * B:(u + 1) * B],
                            start=(u == 0),
                            stop=(u == USH - 1),
                        )
                # hacc += hT/8 - proj_sel_partial
                upd_sb = work.tile([128, FB * B], f32, tag="upd", name=f"up{t}")
                nc.vector.scalar_tensor_tensor(
                    upd_sb, hT_cur, 1.0 / NCORE, pr_ps,
                    op0=ALU.mult, op1=ALU.subtract,
                )
                nc.vector.tensor_add(hacc_sb, hacc_sb, upd_sb)

                if last:
                    break

                # ---- bias + tanh + transpose + AllGather ----
                for c in range(3):
                    nc.tensor.matmul(
                        y_ps[:, c * 512:(c + 1) * 512],
                        lhsT=ohp_sb,
                        rhs=b_sb[:, c * 512:(c + 1) * 512],
                        start=False, stop=True,
                    )
                xf2_sb = work.tile([B, DSH], f32, tag="xf2", name=f"x2{t}")
                nc.scalar.activation(xf2_sb, y_ps, AF.Tanh)
                x2t_ps = ps_x2t.tile([128, NKL * B], f32, tag="x2t", name=f"x2t{t}")
                for ktl in range(NKL):
                    nc.tensor.transpose(
                        x2t_ps[:, ktl * B:(ktl + 1) * B],
                        xf2_sb[:, ktl * 128:(ktl + 1) * 128],
                        ident_sb[0:B, 0:B],
                    )
                x2t_sb = work.tile([128, NKL * B], f32, tag="x2ts", name=f"x2s{t}")
                nc.vector.tensor_copy(x2t_sb, x2t_ps)
                if t < DEPTH - 2:
                    agx_in = dram.tile([128, NKL * B], f32, tag="agxi", name=f"agxi{t}")
                    nc.sync.dma_start(agx_in, x2t_sb)
                    agx_out = dram.tile(
                        [NCORE * 128, NKL * B], f32, tag="agxo", name=f"agxo{t}",
                        addr_space="Shared",
                    )
                    nc.gpsimd.collective_compute(
                        "AllGather", mybir.AluOpType.bypass,
                        replica_groups=RG, ins=[agx_in.opt()], outs=[agx_out.opt()],
                    )
                # hash partials + gather (rides behind the xf AllGather)
                hT_cur = h_partial_and_gather(x2t_sb, h_ps, str(t))
                # next xf (not needed entering the routing-only final step)
                if t < DEPTH - 2:
                    xf_sb = xfp.tile([128, NKT * B], f32, tag="xf", name=f"xf{t + 1}")
                    nc.sync.dma_start(
                        xf_sb, agx_out.rearrange("(r p) c -> p r c", p=128)
                    )

            # ---- epilogue: AllReduce hash accumulator, emit outputs ----
            ar_in = dram.tile([128, FB * B], f32, tag="ari", name="ari")
            nc.sync.dma_start(ar_in, hacc_sb)
            ar_out = dram.tile([128, FB * B], f32, tag="aro", name="aro",
                               addr_space="Shared")
            nc.gpsimd.collective_compute(
                "AllReduce", mybir.AluOpType.add,
                replica_groups=RG, ins=[ar_in.opt()], outs=[ar_out.opt()],
            )
            har_sb = work.tile([128, FB * B], f32, tag="har", name="har")
            nc.sync.dma_start(har_sb, ar_out)
            he_ps_a = ps_small.tile([128, 512], f32, tag="sm", name="he_a")
            he_ps_b = ps_small.tile([128, 512], f32, tag="sm", name="he_b")
            hashes_sb = work.tile([B, HASH], f32, tag="hsh", name="hsh")
            for fb in range(FB):
                ps = he_ps_a if fb < 4 else he_ps_b
                col = (fb % 4) * 128
                nc.tensor.transpose(
                    ps[0:B, col:col + 128],
                    har_sb[:, fb * B:(fb + 1) * B],
                    ident_sb,
                )
            nc.vector.tensor_copy(hashes_sb[:, 0:512], he_ps_a[0:B, :])
            nc.vector.tensor_copy(hashes_sb[:, 512:1024], he_ps_b[0:B, :])
            nc.sync.dma_start(hashes_out, hashes_sb)
            routes_i_sb = work.tile([B, DEPTH], i32, tag="rti", name="rti")
            nc.vector.tensor_copy(routes_i_sb, routes_f_sb)
            nc.sync.dma_start(routes_out, routes_i_sb)
            nc.sync.dma_start(dbg_mags_out, dbg_mags_sb)

    nc.compile()
    return nc


def _get_program():
    global _PROGRAM
    if _PROGRAM is None:
        _PROGRAM = _build_program()
    return _PROGRAM


def _prep_inputs(x, P, basis, Wt, b):
    x = np.asarray(x, dtype=np.float32)
    P = np.asarray(P, dtype=np.float32)
    basis = np.asarray(basis, dtype=np.float32)
    Wt = np.asarray(Wt, dtype=np.float32)
    b = np.asarray(b, dtype=np.float32)

    xf = x.reshape(B, D)
    xT = np.ascontiguousarray(xf.T)                      # (D, B)
    xT_t = np.ascontiguousarray(
        xT.reshape(NKT, 128, B).transpose(1, 0, 2))       # (128, 96, 16)
    iota = np.tile(np.arange(NU, dtype=np.float32), (B, 1))
    ident = np.eye(128, dtype=np.float32)

    in_maps = []
    for r in range(NCORE):
        cs = slice(r * DSH, (r + 1) * DSH)
        wt_r = np.ascontiguousarray(
            Wt[:, cs].reshape(NKP, 2, 128, DSH).transpose(0, 2, 1, 3)
        ).reshape(NKP, 128, 2 * DSH)
        p_r = np.ascontiguousarray(
            P[cs].reshape(NKL, 128, HASH).transpose(1, 0, 2))
        xo_r = np.ascontiguousarray(
            xT[cs].reshape(NKL, 128, B).transpose(1, 0, 2))
        b_r = np.ascontiguousarray(b[:, cs])
        bas_r = basis[r * USH:(r + 1) * USH]              # (4, 1024, 128)
        bas_i = np.ascontiguousarray(
            bas_r.reshape(USH, FB, 128, 128).transpose(2, 0, 1, 3))
        basT_i = np.ascontiguousarray(
            bas_r.transpose(0, 2, 1).reshape(USH, 128, FB, 128).transpose(1, 0, 2, 3))
        g_r = np.einsum("ufk,ufl->ukl", bas_r, bas_r).astype(np.float32)
        g_i = np.ascontiguousarray(g_r.transpose(1, 0, 2))
        usel = np.zeros((NU, USH), dtype=np.float32)
        for ul in range(USH):
            usel[r * USH + ul, ul] = 1.0
        in_maps.append({
            "xT": xT_t, "xo": xo_r, "wt": wt_r, "pp": p_r, "bb": b_r,
            "bas": bas_i, "basT": basT_i, "gg": g_i, "usel": usel,
            "iota": iota, "ident": ident,
        })
    return in_maps


def run(x, P, basis, Wt, b, trace=False):
    from concourse import bass_utils
    nc = _get_program()
    in_maps = _prep_inputs(x, P, basis, Wt, b)
    res = bass_utils.run_bass_kernel_spmd(
        nc, in_maps, core_ids=list(range(NCORE)), trace=trace,
    )
    out = res.results[0]
    global LAST_OUT
    LAST_OUT = {k: v for k, v in out.items() if k.startswith("dbg_")}
    return (out["hashes"], out["routes"]), res


def kernel(x, P, basis, Wt, b, depth=16, **_):
    assert depth == DEPTH
    (hashes, routes), _res = run(x, P, basis, Wt, b, trace=False)
    return hashes, routes


# revision 9
# speedup vs baseline: 1.0794x; 1.0794x over previous
"""HRN routing kernel for Trainium2 (8 NeuronCores, Bass/Tile).

Strategy:
  - Wt (12288x12288, 604MB) is column-sharded across 8 cores. Each step every
    core computes y_shard = xf @ Wt[:, shard] streaming its 75.5MB shard from
    HBM (the memory-bound inner loop), then tanh(+bias), AllGather of the
    transposed activation shard rebuilds the full xf on every core.
  - The hash h = xf2 @ P uses a row-shard of P (resident in SBUF); partial
    hashes are AllGathered and summed locally.
  - Routing (projection magnitudes over 32 units) is sharded 4 units/core;
    the per-unit squared magnitudes use precomputed Gram matrices G_u =
    B_u^T B_u; a tiny AllGather rebuilds the full (16,32) magnitude table on
    every core, argmax/masking is replicated, and the selected-unit residual
    is computed by the owning core only and AllReduced at the end.
  - Step 15 needs routing only (its xf2/h are discarded by the reference).
"""

import numpy as np

B = 16
D = 12288
HASH = 1024
KB = 128
NU = 32
DEPTH = 16
NCORE = 8
DSH = D // NCORE          # 1536 columns of Wt per core
NKT = D // 128            # 96 k-tiles
NKL = DSH // 128          # 12 local k-tiles
NKP = NKT // 2            # 48 paired k-tiles (2 per DMA)
USH = NU // NCORE         # 4 units per core
FB = HASH // 128          # 8 f-blocks

_PROGRAM = None
LAST_OUT = {}


def _build_program():
    import concourse.bass as bass
    import concourse.mybir as mybir
    import concourse.tile as tile
    from concourse import bacc

    f32 = mybir.dt.float32
    i32 = mybir.dt.int32
    AF = mybir.ActivationFunctionType
    ALU = mybir.AluOpType

    nc = bacc.Bacc(
        "TRN2",
        target_bir_lowering=False,
        debug=False,
        enable_asserts=False,
        num_devices=NCORE,
    )

    # ---- I/O ----
    xT_in = nc.dram_tensor("xT", [128, NKT, B], f32, kind="ExternalInput").ap()
    xo_in = nc.dram_tensor("xo", [128, NKL, B], f32, kind="ExternalInput").ap()
    wt_in = nc.dram_tensor("wt", [NKP, 128, 2 * DSH], f32, kind="ExternalInput").ap()
    p_in = nc.dram_tensor("pp", [128, NKL, HASH], f32, kind="ExternalInput").ap()
    b_in = nc.dram_tensor("bb", [NU, DSH], f32, kind="ExternalInput").ap()
    bas_in = nc.dram_tensor("bas", [128, USH, FB, 128], f32, kind="ExternalInput").ap()
    basT_in = nc.dram_tensor("basT", [128, USH, FB, 128], f32, kind="ExternalInput").ap()
    g_in = nc.dram_tensor("gg", [128, USH, 128], f32, kind="ExternalInput").ap()
    usel_in = nc.dram_tensor("usel", [NU, USH], f32, kind="ExternalInput").ap()
    iota_in = nc.dram_tensor("iota", [B, NU], f32, kind="ExternalInput").ap()
    ident_in = nc.dram_tensor("ident", [128, 128], f32, kind="ExternalInput").ap()

    dbg_mags_out = nc.dram_tensor("dbg_mags", [B, DEPTH * NU], f32, kind="ExternalOutput").ap()
    dbg_h0_out = nc.dram_tensor("dbg_h0", [128, FB * B], f32, kind="ExternalOutput").ap()
    hashes_out = nc.dram_tensor("hashes", [B, HASH], f32, kind="ExternalOutput").ap()
    routes_out = nc.dram_tensor("routes", [B, DEPTH], i32, kind="ExternalOutput").ap()

    RG = [list(range(NCORE))]

    with tile.TileContext(nc) as tc:
        with (
            tc.tile_pool(name="const", bufs=1) as cpool,
            tc.tile_pool(name="wtp", bufs=5) as wtp,
            tc.tile_pool(name="xfp", bufs=2) as xfp,
            tc.tile_pool(name="work", bufs=2) as work,
            tc.tile_pool(name="ps_y", bufs=1, space="PSUM") as ps_y,
            tc.tile_pool(name="ps_small", bufs=2, space="PSUM") as ps_small,
            tc.tile_pool(name="ps_x2t", bufs=2, space="PSUM") as ps_x2t,
            tc.tile_pool(name="dram", bufs=2, space="DRAM") as dram,
        ):
            # ---- constants into SBUF ----
            p_sb = cpool.tile([128, NKL * HASH], f32)
            nc.sync.dma_start(p_sb, p_in.rearrange("p a b -> p (a b)"))
            bas_sb = cpool.tile([128, USH * FB * 128], f32)
            nc.sync.dma_start(bas_sb, bas_in.rearrange("p a b c -> p (a b c)"))
            basT_sb = cpool.tile([128, USH * FB * 128], f32)
            nc.sync.dma_start(basT_sb, basT_in.rearrange("p a b c -> p (a b c)"))
            g_sb = cpool.tile([128, USH * 128], f32)
            nc.sync.dma_start(g_sb, g_in.rearrange("p a b -> p (a b)"))
            b_sb = cpool.tile([NU, DSH], f32)
            nc.sync.dma_start(b_sb, b_in)
            usel_sb = cpool.tile([NU, USH], f32)
            nc.sync.dma_start(usel_sb, usel_in)
            iota_sb = cpool.tile([B, NU], f32)
            nc.sync.dma_start(iota_sb, iota_in)
            ident_sb = cpool.tile([128, 128], f32)
            nc.sync.dma_start(ident_sb, ident_in)
            xo_sb = cpool.tile([128, NKL * B], f32)
            nc.sync.dma_start(xo_sb, xo_in.rearrange("p a b -> p (a b)"))
            ones_sb = cpool.tile([128, 1], f32)
            nc.vector.memset(ones_sb, 1.0)
            ones1_sb = cpool.tile([1, 128], f32)
            nc.vector.memset(ones1_sb, 1.0)
            maskacc_sb = cpool.tile([B, NU], f32)
            nc.vector.memset(maskacc_sb, 0.0)
            hacc_sb = cpool.tile([128, FB * B], f32)
            nc.vector.memset(hacc_sb, 0.0)
            routes_f_sb = cpool.tile([B, DEPTH], f32)
            dbg_mags_sb = cpool.tile([B, DEPTH * NU], f32)

            # ---- initial xf ----
            xf_sb = xfp.tile([128, NKT * B], f32, tag="xf", name="xf0")
            nc.sync.dma_start(xf_sb, xT_in.rearrange("p a b -> p (a b)"))

            def h_partial_and_gather(x2t_ap, h_ps, step):
                """96 matmuls: h_partial[f,i] = sum_d P_sh[d,f] * x2t[d,i];
                then AllGather partials and tree-sum into a fresh hT tile."""
                # fb-outer so each fb's accumulation group is consecutive:
                # start=True clears has_written for the WHOLE psum bank, so a
                # later group's start must not interleave an open group.
                for fb in range(FB):
                    for ktl in range(NKL):
                        nc.tensor.matmul(
                            h_ps[:, fb * B:(fb + 1) * B],
                            lhsT=p_sb[:, (ktl * FB + fb) * 128:(ktl * FB + fb + 1) * 128],
                            rhs=x2t_ap[:, ktl * B:(ktl + 1) * B],
                            start=(ktl == 0),
                            stop=(ktl == NKL - 1),
                        )
                hp_sb = work.tile([128, FB * B], f32, tag="hp", name=f"hp{step}")
                nc.vector.tensor_copy(hp_sb, h_ps)
                agh_in = dram.tile([128, FB * B], f32, tag="aghi", name=f"aghi{step}")
                nc.sync.dma_start(agh_in, hp_sb)
                agh_out = dram.tile(
                    [NCORE * 128, FB * B], f32, tag="agho", name=f"agho{step}",
                    addr_space="Shared",
                )
                nc.gpsimd.collective_compute(
                    "AllGather", mybir.AluOpType.bypass,
                    replica_groups=RG, ins=[agh_in.opt()], outs=[agh_out.opt()],
                )
                hr_sb = work.tile([128, NCORE * FB * B], f32, tag="hr", name=f"hr{step}")
                nc.sync.dma_start(hr_sb, agh_out.rearrange("(r p) c -> p r c", p=128))
                hT = work.tile([128, FB * B], f32, tag="hT", name=f"hT{step}")
                nc.vector.tensor_add(hT, hr_sb[:, 0:FB * B], hr_sb[:, FB * B:2 * FB * B])
                for r in range(2, NCORE):
                    nc.vector.tensor_add(
                        hT, hT, hr_sb[:, r * FB * B:(r + 1) * FB * B]
                    )
                return hT

            # ---- h0 (prologue) ----
            sm0 = ps_small.tile([128, 512], f32, tag="sm", name="sm_pro")
            hT_cur = h_partial_and_gather(xo_sb, sm0[:, 356:356 + FB * B], "pro")
            nc.sync.dma_start(dbg_h0_out, hT_cur)

            for t in range(DEPTH):
                last = t == DEPTH - 1
                sm = ps_small.tile([128, 512], f32, tag="sm", name=f"sm{t}")
                c_ps = sm[:, 0:USH * B]
                gc_ps = sm[:, 64:64 + USH * B]
                m2_ps = sm[0:B, 128:128 + USH]
                oh_ps = sm[0:NU, 132:132 + B]
                ow_ps = sm[0:1, 148:148 + USH * B]
                bc_ps = sm[:, 164:164 + USH * B]
                pr_ps = sm[:, 228:228 + FB * B]
                h_ps = sm[:, 356:356 + FB * B]

                if not last:
                    y_ps = ps_y.tile([B, DSH], f32, tag="y", name=f"y{t}")

                def y_chunk(kp0, kp1):
                    for kp in range(kp0, kp1):
                        w_t = wtp.tile([128, 2 * DSH], f32, tag="wt", name=f"w{t}_{kp}")
                        nc.sync.dma_start(w_t, wt_in[kp])
                        for kte in range(2):
                            kt = kp * 2 + kte
                            for c in range(3):
                                nc.tensor.matmul(
                                    y_ps[:, c * 512:(c + 1) * 512],
                                    lhsT=xf_sb[:, kt * B:(kt + 1) * B],
                                    rhs=w_t[:, kte * DSH + c * 512:kte * DSH + (c + 1) * 512],
                                    start=(kt == 0),
                                    stop=False,
                                )

                if not last:
                    y_chunk(0, 16)

                # ---- routing part A: coeff, Gram magnitudes, mags AllGather ----
                for u in range(USH):
                    for fb in range(FB):
                        nc.tensor.matmul(
                            c_ps[:, u * B:(u + 1) * B],
                            lhsT=bas_sb[:, (u * FB + fb) * 128:(u * FB + fb + 1) * 128],
                            rhs=hT_cur[:, fb * B:(fb + 1) * B],
                            start=(fb == 0),
                            stop=(fb == FB - 1),
                        )
                coeff_sb = work.tile([128, USH * B], f32, tag="coeff", name=f"co{t}")
                nc.vector.tensor_copy(coeff_sb, c_ps)
                for u in range(USH):
                    nc.tensor.matmul(
                        gc_ps[:, u * B:(u + 1) * B],
                        lhsT=g_sb[:, u * 128:(u + 1) * 128],
                        rhs=coeff_sb[:, u * B:(u + 1) * B],
                        start=True, stop=True,
                    )
                tmp_sb = work.tile([128, USH * B], f32, tag="tmp", name=f"tm{t}")
                nc.vector.tensor_mul(tmp_sb, coeff_sb, gc_ps)
                for u in range(USH):
                    nc.tensor.matmul(
                        m2_ps[:, u:u + 1],
                        lhsT=tmp_sb[:, u * B:(u + 1) * B],
                        rhs=ones_sb[:, 0:1],
                        start=True, stop=True,
                    )
                m2_sb = work.tile([B, USH], f32, tag="m2", name=f"m2{t}")
                nc.vector.tensor_copy(m2_sb, m2_ps)
                agm_in = dram.tile([B, USH], f32, tag="agmi", name=f"agmi{t}")
                nc.sync.dma_start(agm_in, m2_sb)
                agm_out = dram.tile(
                    [NCORE * B, USH], f32, tag="agmo", name=f"agmo{t}",
                    addr_space="Shared",
                )
                nc.gpsimd.collective_compute(
                    "AllGather", mybir.AluOpType.bypass,
                    replica_groups=RG, ins=[agm_in.opt()], outs=[agm_out.opt()],
                )
                mags_sb = work.tile([B, NU], f32, tag="mags", name=f"mg{t}")
                nc.sync.dma_start(
                    mags_sb, agm_out.rearrange("(r i) u -> i r u", i=B)
                )

                if not last:
                    y_chunk(16, NKP)

                # ---- routing part B: argmax, onehot, residual ----
                nc.vector.tensor_copy(dbg_mags_sb[:, t * NU:(t + 1) * NU], mags_sb)
                masked_sb = work.tile([B, NU], f32, tag="masked", name=f"mk{t}")
                nc.vector.tensor_add(masked_sb, mags_sb, maskacc_sb)
                mx_sb = work.tile([B, 1], f32, tag="mx", name=f"mx{t}")
                nc.vector.reduce_max(mx_sb, masked_sb, axis=mybir.AxisListType.X)
                oh_sb = work.tile([B, NU], f32, tag="oh", name=f"oh{t}")
                nc.vector.tensor_scalar(
                    oh_sb, masked_sb, mx_sb, None, op0=ALU.is_equal
                )
                # maskacc -= 1e30 * onehot
                nc.vector.scalar_tensor_tensor(
                    maskacc_sb, oh_sb, -1e30, maskacc_sb,
                    op0=ALU.mult, op1=ALU.add,
                )
                # routes column t = sum(onehot * iota)
                ridx_sb = work.tile([B, NU], f32, tag="ridx", name=f"ri{t}")
                nc.vector.tensor_mul(ridx_sb, oh_sb, iota_sb)
                nc.vector.reduce_sum(routes_f_sb[:, t:t + 1], ridx_sb, axis=mybir.AxisListType.X)
                # onehotP (32, 16) via PE transpose
                nc.tensor.transpose(oh_ps, oh_sb, ident_sb[0:B, 0:B])
                ohp_sb = work.tile([NU, B], f32, tag="ohp", name=f"op{t}")
                nc.vector.tensor_copy(ohp_sb, oh_ps)
                # own-unit rows of onehotP, each to partition 0: (1, 16) x 4
                for u in range(USH):
                    nc.tensor.matmul(
                        ow_ps[:, u * B:(u + 1) * B],
                        lhsT=usel_sb[:, u:u + 1],
                        rhs=ohp_sb,
                        start=True, stop=True,
                    )
                own_sb = work.tile([1, USH * B], f32, tag="own", name=f"ow{t}")
                nc.vector.tensor_copy(own_sb, ow_ps)
                # broadcast own rows across 128 partitions
                for u in range(USH):
                    nc.tensor.matmul(
                        bc_ps[:, u * B:(u + 1) * B],
                        lhsT=ones1_sb,
                        rhs=own_sb[0:1, u * B:(u + 1) * B],
                        start=True, stop=True,
                    )
                csel_sb = work.tile([128, USH * B], f32, tag="csel", name=f"cs{t}")
                nc.vector.tensor_mul(csel_sb, coeff_sb, bc_ps)
                # selected projection (own units only), accumulated over units
                for fb in range(FB):
                    for u in range(USH):
                        nc.tensor.matmul(
                            pr_ps[:, fb * B:(fb + 1) * B],
                            lhsT=basT_sb[:, (u * FB + fb) * 128:(u * FB + fb + 1) * 128],
                            rhs=csel_sb[:, u * B:(u + 1) * B],
                            start=(u == 0),
                            stop=(u == USH - 1),
                        )
                # hacc += hT/8 - proj_sel_partial
                upd_sb = work.tile([128, FB * B], f32, tag="upd", name=f"up{t}")
                nc.vector.scalar_tensor_tensor(
                    upd_sb, hT_cur, 1.0 / NCORE, pr_ps,
                    op0=ALU.mult, op1=ALU.subtract,
                )
                nc.vector.tensor_add(hacc_sb, hacc_sb, upd_sb)

                if last:
                    break

                # ---- bias + tanh + transpose + AllGather ----
                for c in range(3):
                    nc.tensor.matmul(
                        y_ps[:, c * 512:(c + 1) * 512],
                        lhsT=ohp_sb,
                        rhs=b_sb[:, c * 512:(c + 1) * 512],
                        start=False, stop=True,
                    )
                xf2_sb = work.tile([B, DSH], f32, tag="xf2", name=f"x2{t}")
                nc.scalar.activation(xf2_sb, y_ps, AF.Tanh)
                x2t_ps = ps_x2t.tile([128, NKL * B], f32, tag="x2t", name=f"x2t{t}")
                for ktl in range(NKL):
                    nc.tensor.transpose(
                        x2t_ps[:, ktl * B:(ktl + 1) * B],
                        xf2_sb[:, ktl * 128:(ktl + 1) * 128],
                        ident_sb[0:B, 0:B],
                    )
                x2t_sb = work.tile([128, NKL * B], f32, tag="x2ts", name=f"x2s{t}")
                nc.vector.tensor_copy(x2t_sb, x2t_ps)
                if t < DEPTH - 2:
                    agx_in = dram.tile([128, NKL * B], f32, tag="agxi", name=f"agxi{t}")
                    nc.sync.dma_start(agx_in, x2t_sb)
                    agx_out = dram.tile(
                        [NCORE * 128, NKL * B], f32, tag="agxo", name=f"agxo{t}",
                        addr_space="Shared",
                    )
                    nc.gpsimd.collective_compute(
                        "AllGather", mybir.AluOpType.bypass,
                        replica_groups=RG, ins=[agx_in.opt()], outs=[agx_out.opt()],
                    )
                # hash partials + gather (rides behind the xf AllGather)
                hT_cur = h_partial_and_gather(x2t_sb, h_ps, str(t))
                # next xf (not needed entering the routing-only final step)
                if t < DEPTH - 2:
                    xf_sb = xfp.tile([128, NKT * B], f32, tag="xf", name=f"xf{t + 1}")
                    nc.sync.dma_start(
                        xf_sb, agx_out.rearrange("(r p) c -> p r c", p=128)
                    )

            # ---- epilogue: AllReduce hash accumulator, emit outputs ----
            ar_in = dram.tile([128, FB * B], f32, tag="ari", name="ari")
            nc.sync.dma_start(ar_in, hacc_sb)
            ar_out = dram.tile([128, FB * B], f32, tag="aro", name="aro",
                               addr_space="Shared")
            nc.gpsimd.collective_compute(
                "AllReduce", mybir.AluOpType.add,
                replica_groups=RG, ins=[ar_in.opt()], outs=[ar_out.opt()],
            )
            har_sb = work.tile([128, FB * B], f32, tag="har", name="har")
            nc.sync.dma_start(har_sb, ar_out)
            he_ps_a = ps_small.tile([128, 512], f32, tag="sm", name="he_a")
            he_ps_b = ps_small.tile([128, 512], f32, tag="sm", name="he_b")
            hashes_sb = work.tile([B, HASH], f32, tag="hsh", name="hsh")
            for fb in range(FB):
                ps = he_ps_a if fb < 4 else he_ps_b
                col = (fb % 4) * 128
                nc.tensor.transpose(
                    ps[0:B, col:col + 128],
                    har_sb[:, fb * B:(fb + 1) * B],
                    ident_sb,
                )
            nc.vector.tensor_copy(hashes_sb[:, 0:512], he_ps_a[0:B, :])
            nc.vector.tensor_copy(hashes_sb[:, 512:1024], he_ps_b[0:B, :])
            nc.sync.dma_start(hashes_out, hashes_sb)
            routes_i_sb = work.tile([B, DEPTH], i32, tag="rti", name="rti")
            nc.vector.tensor_copy(routes_i_sb, routes_f_sb)
            nc.sync.dma_start(routes_out, routes_i_sb)
            nc.sync.dma_start(dbg_mags_out, dbg_mags_sb)

    nc.compile()
    return nc


def _get_program():
    global _PROGRAM
    if _PROGRAM is None:
        _PROGRAM = _build_program()
    return _PROGRAM


def _prep_inputs(x, P, basis, Wt, b):
    x = np.asarray(x, dtype=np.float32)
    P = np.asarray(P, dtype=np.float32)
    basis = np.asarray(basis, dtype=np.float32)
    Wt = np.asarray(Wt, dtype=np.float32)
    b = np.asarray(b, dtype=np.float32)

    xf = x.reshape(B, D)
    xT = np.ascontiguousarray(xf.T)                      # (D, B)
    xT_t = np.ascontiguousarray(
        xT.reshape(NKT, 128, B).transpose(1, 0, 2))       # (128, 96, 16)
    iota = np.tile(np.arange(NU, dtype=np.float32), (B, 1))
    ident = np.eye(128, dtype=np.float32)

    in_maps = []
    for r in range(NCORE):
        cs = slice(r * DSH, (r + 1) * DSH)
        wt_r = np.ascontiguousarray(
            Wt[:, cs].reshape(NKP, 2, 128, DSH).transpose(0, 2, 1, 3)
        ).reshape(NKP, 128, 2 * DSH)
        p_r = np.ascontiguousarray(
            P[cs].reshape(NKL, 128, HASH).transpose(1, 0, 2))
        xo_r = np.ascontiguousarray(
            xT[cs].reshape(NKL, 128, B).transpose(1, 0, 2))
        b_r = np.ascontiguousarray(b[:, cs])
        bas_r = basis[r * USH:(r + 1) * USH]              # (4, 1024, 128)
        bas_i = np.ascontiguousarray(
            bas_r.reshape(USH, FB, 128, 128).transpose(2, 0, 1, 3))
        basT_i = np.ascontiguousarray(
            bas_r.transpose(0, 2, 1).reshape(USH, 128, FB, 128).transpose(1, 0, 2, 3))
        g_r = np.einsum("ufk,ufl->ukl", bas_r, bas_r).astype(np.float32)
        g_i = np.ascontiguousarray(g_r.transpose(1, 0, 2))
        usel = np.zeros((NU, USH), dtype=np.float32)
        for ul in range(USH):
            usel[r * USH + ul, ul] = 1.0
        in_maps.append({
            "xT": xT_t, "xo": xo_r, "wt": wt_r, "pp": p_r, "bb": b_r,
            "bas": bas_i, "basT": basT_i, "gg": g_i, "usel": usel,
            "iota": iota, "ident": ident,
        })
    return in_maps


def run(x, P, basis, Wt, b, trace=False):
    from concourse import bass_utils
    nc = _get_program()
    in_maps = _prep_inputs(x, P, basis, Wt, b)
    res = bass_utils.run_bass_kernel_spmd(
        nc, in_maps, core_ids=list(range(NCORE)), trace=trace,
    )
    out = res.results[0]
    global LAST_OUT
    LAST_OUT = {k: v for k, v in out.items() if k.startswith("dbg_")}
    return (out["hashes"], out["routes"]), res


def kernel(x, P, basis, Wt, b, depth=16, **_):
    assert depth == DEPTH
    (hashes, routes), _res = run(x, P, basis, Wt, b, trace=False)
    return hashes, routes
